# revision 2
# baseline (speedup 1.0000x reference)
"""Multi-head self-attention (B=2,S=2048,E=2048,H=16) on 8 trn2 NeuronCores. v11

Sharding: tensor-parallel over heads. Each core owns 2 heads (256 channels):
  - q/k/v projections for its heads only (column-sharded Wq/Wk/Wv)
  - causal attention for its (batch, head) pairs
  - partial output projection (row-sharded Wo); host sums the 8 partials.

Design notes:
  - bf16 activations/weights everywhere (PSUM accumulation stays fp32).
  - v-bias and output bias folded into a host-side constant (bv @ Wo + bo);
    softmax normalize fused into the ctx PSUM drain.
  - denominator: DVE-sums quads of p tiles (bf16 2x), one [128,128]-ones
    matmul per quad whose PSUM result lands pre-broadcast on all partitions.
  - attention loops qb-outer / head-inner with the Wo block matmuls fused
    right after each q-block, giving PE independent work while exp (ACT) runs.
  - ONE psum pool with 8 fixed one-bank tags, opened once: no pool barriers
    between phases, batches, or timing-loop iterations.
  - DMA queues: sync = x-tile prefetch only; scalar/gpsimd = weights + outs.
"""
import sys

sys.path.insert(0, "/opt/trn_rl_repo")
import numpy as np

B, S, E, H = 2, 2048, 2048, 16
D = 128
NCORES = 8
HL = H // NCORES      # heads per core
C = HL * D            # channels per core
BS = B * S
SB = 512              # s-block (projection) / q-block (attention) width
NSB = S // SB         # 4 s-blocks per batch
NET = E // 128        # 16 contraction tiles
NST = S // 128        # 16 s-subtiles per batch

_CACHE = {}


def _build_nc(kloop=None):
    import concourse.mybir as mybir
    import concourse.tile as tile
    from concourse import bacc

    F32 = mybir.dt.float32
    BF16 = mybir.dt.bfloat16
    AF = mybir.ActivationFunctionType
    OP = mybir.AluOpType
    SCALE = 1.0 / float(np.sqrt(D))

    nc = bacc.Bacc(None, target_bir_lowering=False)

    xT_d = nc.dram_tensor("xT", [E, BS], BF16, kind="ExternalInput")
    wq_d = nc.dram_tensor("wq", [E, C], BF16, kind="ExternalInput")
    wk_d = nc.dram_tensor("wk", [E, C], BF16, kind="ExternalInput")
    wv_d = nc.dram_tensor("wv", [E, C], BF16, kind="ExternalInput")
    wo_d = nc.dram_tensor("wo", [C, E], BF16, kind="ExternalInput")
    bq_d = nc.dram_tensor("bq", [HL, D], F32, kind="ExternalInput")
    bk_d = nc.dram_tensor("bk", [HL, D], F32, kind="ExternalInput")
    mk_d = nc.dram_tensor("mk", [128, 128], BF16, kind="ExternalInput")
    out_d = nc.dram_tensor("out", [BS, E], BF16, kind="ExternalOutput")

    with tile.TileContext(nc) as tc:
        with (
            tc.tile_pool(name="const", bufs=1) as cp,
            tc.tile_pool(name="big", bufs=1) as bigp,
            tc.tile_pool(name="xt", bufs=3) as xtp,
            tc.tile_pool(name="pp", bufs=8) as ppool,
            tc.tile_pool(name="work", bufs=2) as wp,
            tc.tile_pool(name="osb", bufs=3) as osp,
            tc.tile_pool(name="ps", bufs=1, space="PSUM") as ps,
        ):
            # ---- constants / weights resident in SBUF ----
            wq_t = cp.tile([128, NET, C], BF16)
            wk_t = cp.tile([128, NET, C], BF16)
            wv_t = cp.tile([128, NET, C], BF16)
            wo_t = cp.tile([128, HL, E], BF16)
            wq_r = wq_d.rearrange("(eo p) c -> p eo c", p=128)
            wk_r = wk_d.rearrange("(eo p) c -> p eo c", p=128)
            wv_r = wv_d.rearrange("(eo p) c -> p eo c", p=128)
            mk_t = cp.tile([128, 128], BF16)
            bq_t = cp.tile([128, HL], F32)
            bk_t = cp.tile([128, HL], F32)
            for h in range(HL):
                nc.scalar.dma_start(bq_t[:, h : h + 1], bq_d[h, :, None])
                nc.scalar.dma_start(bk_t[:, h : h + 1], bk_d[h, :, None])
            # weights stream on scalar/gpsimd queues, earliest-et chunks first;
            # the sync queue is left free so x tiles land immediately
            for eg in range(NET // 4):
                es = slice(eg * 4, (eg + 1) * 4)
                nc.scalar.dma_start(wq_t[:, es, :], wq_r[:, es, :])
                nc.scalar.dma_start(wk_t[:, es, :], wk_r[:, es, :])
                nc.gpsimd.dma_start(wv_t[:, es, :], wv_r[:, es, :])

            ones_f = cp.tile([128, 128], F32)
            nc.vector.memset(ones_f[:], 1.0)
            ones_w = cp.tile([128, 128], BF16)
            nc.vector.tensor_copy(ones_w[:], ones_f[:])

            # 8 one-bank PSUM tags; phases share them via rotation
            def pst(tag):
                return ps.tile([128, SB], F32, tag=tag, name=f"ps_{tag}")

            import contextlib
            if kloop is not None:
                nc.gpsimd.dma_start(mk_t[:], mk_d[:])
                nc.gpsimd.dma_start(wo_t[:], wo_d.rearrange("(co p) e -> p co e", p=128))
                loop_cm = tc.For_i(0, kloop, 1)
            else:
                loop_cm = contextlib.nullcontext()
            with loop_cm:
              for b in range(B):
                  # per-batch activations (bufs=1 -> reused across b)
                  qT = bigp.tile([128, HL, S], BF16, tag="qT")
                  kT = bigp.tile([128, HL, S], BF16, tag="kT")
                  v_t = bigp.tile([128, NST, C], BF16, tag="v")
                  cxT = bigp.tile([128, HL, S], BF16, tag="cxT")

                  # ---------- projections (tags: t0-t3 = q/k, t4/t5 = v) ----------
                  for sb in range(NSB):
                      s0 = sb * SB
                      qps = [pst(f"t{h}") for h in range(HL)]
                      kps = [pst(f"t{2 + h}") for h in range(HL)]
                      xt = xtp.tile([128, NET, SB], BF16, tag="xt")
                      for eg in range(NET // 4):
                          nc.sync.dma_start(
                              xt[:, eg * 4 : (eg + 1) * 4, :],
                              xT_d.rearrange("(eo p) s -> p eo s", p=128)[
                                  :, eg * 4 : (eg + 1) * 4, b * S + s0 : b * S + s0 + SB
                              ],
                          )
                          for ei in range(4):
                              et = eg * 4 + ei
                              st_flags = dict(start=(et == 0), stop=(et == NET - 1))
                              for h in range(HL):
                                  hs = slice(h * D, (h + 1) * D)
                                  nc.tensor.matmul(qps[h][:], wq_t[:, et, hs], xt[:, et, :], **st_flags)
                                  nc.tensor.matmul(kps[h][:], wk_t[:, et, hs], xt[:, et, :], **st_flags)
                      for h in range(HL):
                          nc.scalar.activation(
                              qT[:, h, s0 : s0 + SB], qps[h][:], AF.Identity,
                              bias=bq_t[:, h : h + 1],
                          )
                          # k drains on DVE to keep ACT free for attention exp
                          nc.vector.tensor_scalar_add(
                              kT[:, h, s0 : s0 + SB], kps[h][:], bk_t[:, h : h + 1]
                          )
                      for vh in range(2):
                          vps = [pst(f"t{4 + j}") for j in range(2)]
                          for et in range(NET):
                              st_flags = dict(start=(et == 0), stop=(et == NET - 1))
                              for st in range(2):
                                  nc.tensor.matmul(
                                      vps[st][:, :C],
                                      xt[:, et, (vh * 2 + st) * 128 : (vh * 2 + st + 1) * 128],
                                      wv_t[:, et, :],
                                      **st_flags,
                                  )
                          for st in range(2):
                              nc.vector.tensor_copy(v_t[:, sb * 4 + vh * 2 + st, :], vps[st][:, :C])

                  if b == 0 and kloop is None:
                      # late const loads: needed only from attention phase on
                      nc.scalar.dma_start(mk_t[:], mk_d[:])
                      nc.scalar.dma_start(wo_t[:], wo_d.rearrange("(co p) e -> p co e", p=128))

                  # ---------- causal attention + fused output projection ----------
                  # tags: scores t0-t2 (rotate), ctx t3/t4, den t5, wo o0/o1
                  nhq = 0
                  for qb in range(NSB):
                      q0 = qb * SB
                      nkt = (qb + 1) * (SB // 128)
                      for h in range(HL):
                          hs = slice(h * D, (h + 1) * D)
                          ctxps = pst(f"t{3 + nhq % 2}")
                          dps = pst("t5")
                          nhq += 1
                          acc, off0 = None, 0
                          for kt in range(nkt):
                              j = kt - (nkt - SB // 128)
                              off = 128 * j if j > 0 else 0  # fully-masked cols skipped
                              # p/scores columns aligned to absolute q within the block
                              sps = pst(f"t{kt % 3}")
                              nc.tensor.matmul(
                                  sps[:, off:SB],
                                  kT[:, h, kt * 128 : (kt + 1) * 128],
                                  qT[:, h, q0 + off : q0 + SB],
                                  start=True, stop=True,
                              )
                              p = ppool.tile([128, SB], BF16, tag="p")
                              nc.scalar.activation(p[:, off:SB], sps[:, off:SB], AF.Exp, scale=SCALE)
                              if j >= 0:
                                  # triangle block = first 128 live columns
                                  nc.vector.tensor_tensor(
                                      p[:, off : off + 128], p[:, off : off + 128],
                                      mk_t[:], OP.mult,
                                  )
                              nc.tensor.matmul(
                                  ctxps[:, off:SB], v_t[:, kt, hs], p[:, off:SB],
                                  start=(kt == 0), stop=(kt == nkt - 1),
                              )
                              # denominator: DVE-sums quads of p tiles (bf16 2x),
                              # one ones-matmul per quad on the PE
                              if kt % 4 == 0:
                                  acc, off0 = p, off
                              else:
                                  nc.vector.tensor_tensor(
                                      acc[:, off:SB], acc[:, off:SB], p[:, off:SB], OP.add
                                  )
                              if kt % 4 == 3:
                                  nc.tensor.matmul(
                                      dps[:, off0:SB], ones_w[:], acc[:, off0:SB],
                                      start=(kt == 3), stop=(kt == nkt - 1),
                                  )
                          bt = wp.tile([128, SB], F32, tag="B")
                          nc.vector.reciprocal(bt[:], dps[:])
                          # normalize fused into the PSUM drain (v-bias folded host-side)
                          nc.vector.tensor_tensor(
                              cxT[:, h, q0 : q0 + SB], ctxps[:], bt[:], OP.mult
                          )

                      # -- output projection for this q-block (both heads ready) --
                      for qt in range(qb * 4, (qb + 1) * 4):
                          osb = osp.tile([128, E], BF16, tag="osb")
                          for eb in range(E // SB):
                              ops = pst(f"o{(qt * 4 + eb) % 2}")
                              for h in range(HL):
                                  nc.tensor.matmul(
                                      ops[:],
                                      cxT[:, h, qt * 128 : (qt + 1) * 128],
                                      wo_t[:, h, eb * SB : (eb + 1) * SB],
                                      start=(h == 0), stop=(h == HL - 1),
                                  )
                              dst = osb[:, eb * SB : (eb + 1) * SB]
                              if eb % 4 == 0:
                                  nc.scalar.copy(dst, ops[:])
                              else:
                                  nc.vector.tensor_copy(dst, ops[:])
                          dma_eng = nc.sync if qt % 2 == 0 else nc.gpsimd
                          dma_eng.dma_start(
                              out_d[b * S + qt * 128 : b * S + (qt + 1) * 128, :], osb[:]
                          )

    nc.compile()
    return nc


def make_in_maps(x, Wq, bq, Wk, bk, Wv, bv, Wo, bo):
    import ml_dtypes
    bf16 = ml_dtypes.bfloat16
    xT = np.ascontiguousarray(np.asarray(x, np.float32).reshape(BS, E).T.astype(bf16))
    ki = np.arange(128)[:, None]
    qi = np.arange(128)[None, :]
    masks = (ki <= qi).astype(bf16)
    in_maps = []
    for c in range(NCORES):
        ch = slice(c * C, (c + 1) * C)
        in_maps.append(
            {
                "xT": xT,
                "wq": np.ascontiguousarray(np.asarray(Wq, np.float32)[ch, :].T.astype(bf16)),
                "wk": np.ascontiguousarray(np.asarray(Wk, np.float32)[ch, :].T.astype(bf16)),
                "wv": np.ascontiguousarray(np.asarray(Wv, np.float32)[ch, :].T.astype(bf16)),
                "wo": np.ascontiguousarray(np.asarray(Wo, np.float32)[:, ch].T.astype(bf16)),
                "bq": np.asarray(bq, np.float32)[ch].reshape(HL, D),
                "bk": np.asarray(bk, np.float32)[ch].reshape(HL, D),
                "mk": masks,
            }
        )
    return in_maps


def get_nc(kloop=None):
    key = ("nc", kloop)
    if key not in _CACHE:
        _CACHE[key] = _build_nc(kloop)
    return _CACHE[key]


def kernel(x, Wq, bq, Wk, bk, Wv, bv, Wo, bo):
    from concourse.bass_utils import run_bass_kernel_spmd

    nc = get_nc()
    in_maps = make_in_maps(x, Wq, bq, Wk, bk, Wv, bv, Wo, bo)
    res = run_bass_kernel_spmd(nc, in_maps, core_ids=list(range(NCORES)))
    acc = np.zeros((BS, E), np.float64)
    for r in res.results:
        acc += r["out"].astype(np.float64)
    # host-folded biases: bo + sum_c bv_c @ Wo[:, ch_c].T  (== Wo @ bv + bo)
    acc += (np.asarray(Wo, np.float64) @ np.asarray(bv, np.float64))[None, :]
    acc += np.asarray(bo, np.float64)[None, :]
    return acc.astype(np.float32).reshape(B, S, E)


if __name__ == "__main__":
    import reference

    inputs = {k: np.asarray(v) for k, v in reference.setup_inputs().items()}
    expected = np.asarray(reference.reference(**inputs))
    actual = kernel(**inputs)
    err = np.linalg.norm(actual - expected) / np.linalg.norm(expected)
    print("Relative error:", err)
